# revision 23
# baseline (speedup 1.0000x reference)
"""MoE expert-group kernel for Trainium2 (8 NeuronCores).

Problem: T=2048 tokens, E=8 experts, D=1024, I=2048.
  out[t] = silu(x[t] @ w_gate[e]) * (x[t] @ w_up[e]) @ w_down[e],  e = expert_indices[t]

Strategy: expert parallelism. Host-side (numpy) routing gathers tokens by
expert (this is the "all-to-all"); core e runs expert e's dense
gate/up/silu/down pipeline; host scatters rows back.

On-chip formulation is fully transposed so no transposes are ever needed:
  gateT = Wg^T @ X^T        (stationary = 128x128 Wg block, moving = xT [128, C])
  hidT  = silu(gateT)*upT   (ACT Silu + one DVE mul, written bf16)
  outT  = Wd^T @ hidT       (stationary = 128x128 Wd block, moving = hT [128, C])

Numerics: weights are shipped as int8 (per-expert linear quantization,
scale = absmax/127) and cast to bf16 *inside* the gpsimd DMA engines, so
HBM weight traffic halves (6.3MB/core) while the PE still runs plain bf16
matmuls on exact integer values. x stays bf16. The dequant scales ride in
a tiny per-core consts tensor: silu applies s_g as the ACT pre-scale, and
the final PSUM->SBUF copy (ACT Copy) applies s_u*s_d. Measured end-to-end
rel err ~8e-3 (vs 4.8e-3 all-bf16, 2e-2 budget).

DMA design (what actually matters on TRN2):
- The PE consumes weights at ~132GB/s (int8 bytes) against a ~358GB/s HBM
  port, so a single casting queue feeds the whole weight stream with 2x+
  slack; queue alignment is no longer fragile (it was at bf16: 264GB/s
  consumption vs ~330 delivered across 3 contending queues).
- gpsimd (SWDGE, the only ring that can cast): the entire weight stream in
  consumption order - single-slice gate/up blocks for i0-i1 to bootstrap,
  then 2-slice blocks, then the 8 d-major wd chunks.
- sync (HWDGE): x (two halves, so the first matmuls gate on only half),
  consts, then the 8 output strips.
- scalar issues no DMAs: its instruction stream carries the 16 Silu ops in
  phase 1 and the 8 scaled output copies in phase 2.
- wd is packed d-major so each output d-strip's weights are one contiguous
  0.5MB chunk; phase 2 (dd-major) can start on chunk 0 without waiting for
  the full down-projection.
"""

import sys

import numpy as np

try:
    import concourse  # noqa: F401
except ImportError:  # grading env fallback
    sys.path.insert(0, "/opt/trn_rl_repo")

import ml_dtypes

T, E, D, I = 2048, 8, 1024, 2048
ND = D // 128  # 8 contraction tiles for gate/up
NI = I // 128  # 16 contraction tiles for down

_PROGRAM_CACHE = {}


def _build_program(C):
    """Build + compile the per-core Bass program for token capacity C."""
    import concourse.bass as bass  # noqa: F401
    import concourse.mybir as mybir
    import concourse.tile as tile
    from concourse import bacc

    BF = mybir.dt.bfloat16
    I8 = mybir.dt.int8
    F32 = mybir.dt.float32

    nc = bacc.Bacc(
        "TRN2",
        target_bir_lowering=False,
        debug=False,
        num_devices=E,
        enable_partition_id=False,
    )
    # xT packed: [128, ND*C], partition p / slot d*C+c  <-  x[tok c, d*128+p]
    xT_d = nc.dram_tensor("xT", [128, ND * C], BF, kind="ExternalInput").ap()
    # wg/wu packed: [128, NI*D] int8, free slot i*D + d*128 + q  <-
    #   round(w[d*128+p, i*128+q]/s) for the [D, I] projections
    wg_d = nc.dram_tensor("wg", [128, NI * D], I8, kind="ExternalInput").ap()
    wu_d = nc.dram_tensor("wu", [128, NI * D], I8, kind="ExternalInput").ap()
    # wd packed d-major: [128, ND*I] int8, free slot d*I + i*128 + q  <-
    #   round(w[i*128+p, d*128+q]/s) for the [I, D] projection
    wd_d = nc.dram_tensor("wd", [128, NI * D], I8, kind="ExternalInput").ap()
    # consts[:, 0] = s_g (gate dequant scale), consts[:, 1] = s_u*s_d
    cst_d = nc.dram_tensor("cst", [128, 2], F32, kind="ExternalInput").ap()
    outT_d = nc.dram_tensor("outT", [D, C], F32, kind="ExternalOutput").ap()

    # PSUM bank holds 2KB/partition = 512 fp32: split the moving dim if needed.
    n_chunks = -(-C // 512)
    chunks = [(n * 512, min(512, C - n * 512)) for n in range(n_chunks)]

    with tile.TileContext(nc) as tc:
        with (
            tc.tile_pool(name="xp", bufs=1) as xp,
            tc.tile_pool(name="cp", bufs=1) as cp,
            tc.tile_pool(name="wp", bufs=1) as wp,
            tc.tile_pool(name="hp", bufs=1) as hp,
            tc.tile_pool(name="sp", bufs=3) as sp,
            tc.tile_pool(name="op", bufs=3) as op,
            tc.tile_pool(name="pg", bufs=3, space="PSUM") as pg,
            tc.tile_pool(name="pu", bufs=3, space="PSUM") as pu,
            tc.tile_pool(name="po", bufs=2, space="PSUM") as po,
        ):
            # x halves on two parallel rings (sync + scalar) so both are in
            # flight at once; serial splits on one ring just stack ~0.6us
            # completion receipts. cst (tiny, needed only by the first Silu
            # at ~13us) rides behind x-lo.
            xT = xp.tile([128, ND * C], BF, tag="x", name="xT")
            half = (ND // 2) * C
            nc.sync.dma_start(xT[:, bass.ds(0, half)], xT_d[:, bass.ds(0, half)])
            nc.scalar.dma_start(
                xT[:, bass.ds(half, half)], xT_d[:, bass.ds(half, half)]
            )
            cst = cp.tile([128, 2], F32, tag="c", name="cst")
            nc.sync.dma_start(cst[:], cst_d[:, :])

            # PE warm-up: ~3us of junk matmuls during the bootstrap DMA
            # window, so the HAM clock-gate reaches 8/8 before the first
            # real matmul (otherwise the first ~3.4us of real work runs at
            # 1.2GHz). Zeros from a memset tile; results land in a scratch
            # rotation slot of the g PSUM pool and are never read.
            wj = cp.tile([128, 128], BF, tag="wj", name="w_junk")
            nc.vector.memset(wj[:], 0)
            warm_ps = pg.tile([128, chunks[0][1]], F32, tag="g", name="warm_ps")
            for _ in range(70):
                nc.tensor.matmul(
                    warm_ps[:, bass.ds(0, 64)],
                    wj[:],
                    wj[:, bass.ds(0, 64)],
                    start=True,
                    stop=True,
                )

            # Whole weight stream on the gpsimd casting ring, in exact
            # phase-1 consumption order (g0 u0 g1 u1 g23 u23 ...): the ring
            # is FIFO, so delivery matches need-order by construction.
            stream = [
                ("g", (0, 1)), ("u", (0, 1)), ("g", (1, 1)), ("u", (1, 1)),
            ]
            for k in range(1, NI // 2):
                stream.append(("g", (2 * k, 2)))
                stream.append(("u", (2 * k, 2)))
            src = {"g": wg_d, "u": wu_d}
            smap = {"g": [None] * NI, "u": [None] * NI}
            for n, (proj, (b0, nb)) in enumerate(stream):
                t = wp.tile(
                    [128, nb * D], BF, tag=f"w{proj}{b0}", name=f"w{proj}{b0}"
                )
                if n == 4:
                    # Gate the 2-slice blocks (needed only from ~16us) and
                    # everything FIFO behind them on x completion: the cast
                    # stream writes 2x its bytes to SBUF and otherwise
                    # starves the x bootstrap to ~78GB/s. The 2-col copy
                    # spans both x-half DMAs (RAW) and this block's tile
                    # (WAW), so the ring pauses until x is fully resident.
                    nc.vector.tensor_copy(
                        t[:, bass.ds(0, 2)], xT[:, bass.ds(half - 1, 2)]
                    )
                nc.gpsimd.dma_start(t[:], src[proj][:, bass.ds(b0 * D, nb * D)])
                for i in range(b0, b0 + nb):
                    smap[proj][i] = (t, i - b0)

            def wslice(proj, i, d):
                t, loc = smap[proj][i]
                return t[:, bass.ds(loc * D + d * 128, 128)]

            # wd chunks ride at the stream tail in dd order.
            wd_t = []
            for dd in range(ND):
                t = wp.tile([128, I], BF, tag=f"wd{dd}", name=f"wd{dd}")
                nc.gpsimd.dma_start(t[:], wd_d[:, bass.ds(dd * I, I)])
                wd_t.append(t)

            def wdslice(i, dd):
                return wd_t[dd][:, bass.ds(i * 128, 128)]

            # Phase 1: hidT[i] = silu(Wg^T x^T) * (Wu^T x^T), one 128-row
            # strip of the intermediate dim per iteration. The matmuls see
            # dequant-scaled integers; ACT Silu folds s_g back in via its
            # pre-scale, so s_sb is the true silu(gate) and hT carries only
            # the 1/s_u factor (descaled at the output copy).
            hT = []
            for i in range(NI):
                h_t = hp.tile([128, C], BF, tag=f"h{i}", name=f"hT{i}")
                for c0, cn in chunks:
                    csl = bass.ds(c0, cn)
                    g_ps = pg.tile([128, cn], F32, tag="g", name="g_ps")
                    u_ps = pu.tile([128, cn], F32, tag="u", name="u_ps")
                    for d in range(ND):
                        xsl = bass.ds(d * C + c0, cn)
                        nc.tensor.matmul(
                            g_ps[:],
                            wslice("g", i, d),
                            xT[:, xsl],
                            start=(d == 0),
                            stop=(d == ND - 1),
                        )
                    for d in range(ND):
                        xsl = bass.ds(d * C + c0, cn)
                        nc.tensor.matmul(
                            u_ps[:],
                            wslice("u", i, d),
                            xT[:, xsl],
                            start=(d == 0),
                            stop=(d == ND - 1),
                        )
                    s_sb = sp.tile([128, cn], F32, tag="s", name="s_sb")
                    nc.scalar.activation(
                        s_sb[:],
                        g_ps[:],
                        mybir.ActivationFunctionType.Silu,
                        scale=cst[:, bass.ds(0, 1)],
                    )
                    nc.vector.tensor_mul(h_t[:, csl], s_sb[:], u_ps[:])
                hT.append(h_t)

            # Phase 2: outT[dstrip] = Wd^T @ hidT, accumulated over all 16
            # intermediate strips. The PSUM->SBUF copy runs on ACT (idle in
            # phase 2) and applies the s_u*s_d dequant in its scale.
            for dd in range(ND):
                dsl = bass.ds(dd * 128, 128)
                for c0, cn in chunks:
                    csl = bass.ds(c0, cn)
                    o_ps = po.tile([128, cn], F32, tag="o", name="o_ps")
                    for i in range(NI):
                        nc.tensor.matmul(
                            o_ps[:],
                            wdslice(i, dd),
                            hT[i][:, csl],
                            start=(i == 0),
                            stop=(i == NI - 1),
                        )
                    o_sb = op.tile([128, cn], F32, tag="ob", name="o_sb")
                    if dd < ND - 1:
                        nc.vector.tensor_scalar_mul(
                            o_sb[:], o_ps[:], cst[:, bass.ds(1, 1)]
                        )
                        nc.sync.dma_start(outT_d[dsl, csl], o_sb[:])
                    else:
                        # final strip: copy + drain in two halves on parallel
                        # rings so the post-last-matmul tail is half as long
                        ch = cn // 2
                        h0, h1 = bass.ds(c0, ch), bass.ds(c0 + ch, cn - ch)
                        nc.vector.tensor_scalar_mul(
                            o_sb[:, bass.ds(0, ch)],
                            o_ps[:, bass.ds(0, ch)],
                            cst[:, bass.ds(1, 1)],
                        )
                        nc.scalar.dma_start(
                            outT_d[dsl, h0], o_sb[:, bass.ds(0, ch)]
                        )
                        nc.vector.tensor_scalar_mul(
                            o_sb[:, bass.ds(ch, cn - ch)],
                            o_ps[:, bass.ds(ch, cn - ch)],
                            cst[:, bass.ds(1, 1)],
                        )
                        nc.sync.dma_start(outT_d[dsl, h1], o_sb[:, bass.ds(ch, cn - ch)])

    nc.compile()
    return nc


def _get_program(C):
    if C not in _PROGRAM_CACHE:
        _PROGRAM_CACHE[C] = _build_program(C)
    return _PROGRAM_CACHE[C]


def _run(nc, in_maps, trace=False):
    from concourse.bass_utils import run_bass_kernel_spmd

    return run_bass_kernel_spmd(nc, in_maps, core_ids=list(range(E)), trace=trace)


def _quant8(w):
    # per-expert-tensor linear int8 quantization; returns (q, scale)
    s = float(np.abs(w).max()) / 127.0
    if s == 0.0:
        s = 1.0
    q = np.clip(np.rint(w / s), -127, 127).astype(np.int8)
    return q, s


def _pack_w(q, transpose):
    # transpose=True (wg/wu, [D, I]): -> [128, NI*D], free slot i*D + d*128 + q,
    #   block (i,d) = w[d*128:+128, i*128:+128]
    # transpose=False (wd, [I, D]): -> [128, ND*I] d-major, free slot
    #   d*I + i*128 + q, block (i,d) = w[i*128:+128, d*128:+128]
    if transpose:
        b = q.reshape(ND, 128, NI, 128).transpose(1, 2, 0, 3)  # p, i, d, q
    else:
        b = q.reshape(NI, 128, ND, 128).transpose(1, 2, 0, 3)  # p, d, i, q
    return np.ascontiguousarray(b.reshape(128, NI * D))


def _kernel_numpy(x, idx, w_gate, w_up, w_down):
    # exact fallback for pathological token skew (SBUF can't hold >~1536
    # tokens per expert); normal inputs never take this path
    out = np.zeros((T, D), dtype=np.float32)
    for e in range(E):
        m = idx == e
        if not m.any():
            continue
        g = x[m] @ w_gate[e]
        u = x[m] @ w_up[e]
        out[m] = (g / (1.0 + np.exp(-g)) * u) @ w_down[e]
    return out


def kernel(x, expert_indices, w_gate, w_up, w_down, _trace=False, _results=None):
    x = np.asarray(x)
    idx = np.asarray(expert_indices).astype(np.int64)
    counts = np.bincount(idx, minlength=E)
    C = int(max(128, -(-counts.max() // 4) * 4))
    if C > 1536:
        return _kernel_numpy(
            x, idx, np.asarray(w_gate), np.asarray(w_up), np.asarray(w_down)
        )

    nc = _get_program(C)

    order = np.argsort(idx, kind="stable")
    starts = np.zeros(E + 1, dtype=np.int64)
    np.cumsum(counts, out=starts[1:])

    bf16 = ml_dtypes.bfloat16
    in_maps = []
    for e in range(E):
        toks = order[starts[e] : starts[e + 1]]
        # xT packed: [128, ND*C]; [p, d*C+c] = x[tok c, d*128+p]
        xTg = np.zeros((128, ND, C), dtype=bf16)
        xTg[:, :, : len(toks)] = (
            x[toks].astype(bf16).T.reshape(ND, 128, len(toks)).transpose(1, 0, 2)
        )
        qg, sg = _quant8(np.asarray(w_gate[e]))
        qu, su = _quant8(np.asarray(w_up[e]))
        qd, sd = _quant8(np.asarray(w_down[e]))
        cst = np.empty((128, 2), dtype=np.float32)
        cst[:, 0] = sg
        cst[:, 1] = su * sd
        in_maps.append(
            {
                "xT": xTg.reshape(128, ND * C),
                "wg": _pack_w(qg, True),
                "wu": _pack_w(qu, True),
                "wd": _pack_w(qd, False),
                "cst": cst,
            }
        )

    res = _run(nc, in_maps, trace=_trace)
    if _results is not None:
        _results.append(res)

    out = np.zeros((T, D), dtype=np.float32)
    for e in range(E):
        toks = order[starts[e] : starts[e + 1]]
        outT = res.results[e]["outT"]  # [D, C] fp32
        out[toks] = outT[:, : len(toks)].T
    return out


# revision 25
# speedup vs baseline: 1.0428x; 1.0428x over previous
"""MoE expert-group kernel for Trainium2 (8 NeuronCores).

Problem: T=2048 tokens, E=8 experts, D=1024, I=2048.
  out[t] = silu(x[t] @ w_gate[e]) * (x[t] @ w_up[e]) @ w_down[e],  e = expert_indices[t]

Strategy: expert parallelism. Host-side (numpy) routing gathers tokens by
expert (this is the "all-to-all"); core e runs expert e's dense
gate/up/silu/down pipeline; host scatters rows back.

On-chip formulation is fully transposed so no transposes are ever needed:
  gateT = Wg^T @ X^T        (stationary = 128x128 Wg block, moving = xT [128, C])
  hidT  = silu(gateT)*upT   (ACT Silu + one DVE mul, written bf16)
  outT  = Wd^T @ hidT       (stationary = 128x128 Wd block, moving = hT [128, C])

Numerics: weights are shipped as int8 (per-expert linear quantization,
scale = absmax/127) and cast to bf16 *inside* the gpsimd DMA engines, so
HBM weight traffic halves (6.3MB/core) while the PE still runs plain bf16
matmuls on exact integer values. x stays bf16. The dequant scales ride in
a tiny per-core consts tensor: silu applies s_g as the ACT pre-scale, and
the final PSUM->SBUF copy (ACT Copy) applies s_u*s_d. Measured end-to-end
rel err ~8e-3 (vs 4.8e-3 all-bf16, 2e-2 budget).

DMA design (what actually matters on TRN2):
- The PE consumes weights at ~132GB/s (int8 bytes) against a ~358GB/s HBM
  port, so a single casting queue feeds the whole weight stream with 2x+
  slack; queue alignment is no longer fragile (it was at bf16: 264GB/s
  consumption vs ~330 delivered across 3 contending queues).
- gpsimd (SWDGE, the only ring that can cast): the entire weight stream in
  consumption order - single-slice gate/up blocks for i0-i1 to bootstrap,
  then 2-slice blocks, then the 8 d-major wd chunks.
- sync (HWDGE): x (two halves, so the first matmuls gate on only half),
  consts, then the 8 output strips.
- scalar issues no DMAs: its instruction stream carries the 16 Silu ops in
  phase 1 and the 8 scaled output copies in phase 2.
- wd is packed d-major so each output d-strip's weights are one contiguous
  0.5MB chunk; phase 2 (dd-major) can start on chunk 0 without waiting for
  the full down-projection.
"""

import sys

import numpy as np

try:
    import concourse  # noqa: F401
except ImportError:  # grading env fallback
    sys.path.insert(0, "/opt/trn_rl_repo")

import ml_dtypes

T, E, D, I = 2048, 8, 1024, 2048
ND = D // 128  # 8 contraction tiles for gate/up
NI = I // 128  # 16 contraction tiles for down

_PROGRAM_CACHE = {}


def _build_program(C):
    """Build + compile the per-core Bass program for token capacity C."""
    import concourse.bass as bass  # noqa: F401
    import concourse.mybir as mybir
    import concourse.tile as tile
    from concourse import bacc

    BF = mybir.dt.bfloat16
    I8 = mybir.dt.int8
    F32 = mybir.dt.float32

    nc = bacc.Bacc(
        "TRN2",
        target_bir_lowering=False,
        debug=False,
        num_devices=E,
        enable_partition_id=False,
    )
    # xT packed: [128, ND*C], partition p / slot d*C+c  <-  x[tok c, d*128+p]
    xT_d = nc.dram_tensor("xT", [128, ND * C], BF, kind="ExternalInput").ap()
    # wg/wu packed: [128, NI*D] int8, free slot i*D + d*128 + q  <-
    #   round(w[d*128+p, i*128+q]/s) for the [D, I] projections
    wg_d = nc.dram_tensor("wg", [128, NI * D], I8, kind="ExternalInput").ap()
    wu_d = nc.dram_tensor("wu", [128, NI * D], I8, kind="ExternalInput").ap()
    # wd packed d-major: [128, ND*I] int8, free slot d*I + i*128 + q  <-
    #   round(w[i*128+p, d*128+q]/s) for the [I, D] projection
    wd_d = nc.dram_tensor("wd", [128, NI * D], I8, kind="ExternalInput").ap()
    # consts[:, 0] = s_g (gate dequant scale), consts[:, 1] = s_u*s_d
    cst_d = nc.dram_tensor("cst", [128, 2], F32, kind="ExternalInput").ap()
    outT_d = nc.dram_tensor("outT", [D, C], F32, kind="ExternalOutput").ap()

    # PSUM bank holds 2KB/partition = 512 fp32: split the moving dim if needed.
    n_chunks = -(-C // 512)
    chunks = [(n * 512, min(512, C - n * 512)) for n in range(n_chunks)]

    with tile.TileContext(nc) as tc:
        with (
            tc.tile_pool(name="xp", bufs=1) as xp,
            tc.tile_pool(name="cp", bufs=1) as cp,
            tc.tile_pool(name="wp", bufs=1) as wp,
            tc.tile_pool(name="hp", bufs=1) as hp,
            tc.tile_pool(name="sp", bufs=3) as sp,
            tc.tile_pool(name="op", bufs=3) as op,
            tc.tile_pool(name="pg", bufs=3, space="PSUM") as pg,
            tc.tile_pool(name="pu", bufs=3, space="PSUM") as pu,
            tc.tile_pool(name="po", bufs=2, space="PSUM") as po,
        ):
            # x halves on two parallel rings (sync + scalar) so both are in
            # flight at once; serial splits on one ring just stack ~0.6us
            # completion receipts. cst (tiny, needed only by the first Silu
            # at ~13us) rides behind x-lo.
            xT = xp.tile([128, ND * C], BF, tag="x", name="xT")
            half = (ND // 2) * C
            nc.sync.dma_start(xT[:, bass.ds(0, half)], xT_d[:, bass.ds(0, half)])
            nc.scalar.dma_start(
                xT[:, bass.ds(half, half)], xT_d[:, bass.ds(half, half)]
            )
            cst = cp.tile([128, 2], F32, tag="c", name="cst")
            nc.sync.dma_start(cst[:], cst_d[:, :])

            # PE warm-up: ~3us of junk matmuls during the bootstrap DMA
            # window, so the HAM clock-gate reaches 8/8 before the first
            # real matmul (otherwise the first ~3.4us of real work runs at
            # 1.2GHz). Zeros from a memset tile; results land in a scratch
            # rotation slot of the g PSUM pool and are never read.
            wj = cp.tile([128, 128], BF, tag="wj", name="w_junk")
            nc.vector.memset(wj[:], 0)
            warm_ps = pg.tile([128, chunks[0][1]], F32, tag="g", name="warm_ps")
            for _ in range(80):
                nc.tensor.matmul(
                    warm_ps[:, bass.ds(0, 64)],
                    wj[:],
                    wj[:, bass.ds(0, 64)],
                    start=True,
                    stop=True,
                )

            # Whole weight stream on the gpsimd casting ring, in exact
            # phase-1 consumption order (g0 u0 g1 u1 g23 u23 ...): the ring
            # is FIFO, so delivery matches need-order by construction.
            stream = [
                ("g", (0, 1)), ("u", (0, 1)), ("g", (1, 1)), ("u", (1, 1)),
            ]
            for k in range(1, NI // 2):
                stream.append(("g", (2 * k, 2)))
                stream.append(("u", (2 * k, 2)))
            src = {"g": wg_d, "u": wu_d}
            smap = {"g": [None] * NI, "u": [None] * NI}
            for proj, (b0, nb) in stream:
                t = wp.tile(
                    [128, nb * D], BF, tag=f"w{proj}{b0}", name=f"w{proj}{b0}"
                )
                nc.gpsimd.dma_start(t[:], src[proj][:, bass.ds(b0 * D, nb * D)])
                for i in range(b0, b0 + nb):
                    smap[proj][i] = (t, i - b0)

            def wslice(proj, i, d):
                t, loc = smap[proj][i]
                return t[:, bass.ds(loc * D + d * 128, 128)]

            # wd chunks ride at the stream tail in dd order.
            wd_t = []
            for dd in range(ND):
                t = wp.tile([128, I], BF, tag=f"wd{dd}", name=f"wd{dd}")
                nc.gpsimd.dma_start(t[:], wd_d[:, bass.ds(dd * I, I)])
                wd_t.append(t)

            def wdslice(i, dd):
                return wd_t[dd][:, bass.ds(i * 128, 128)]

            # Phase 1: hidT[i] = silu(Wg^T x^T) * (Wu^T x^T), one 128-row
            # strip of the intermediate dim per iteration. The matmuls see
            # dequant-scaled integers; ACT Silu folds s_g back in via its
            # pre-scale, so s_sb is the true silu(gate) and hT carries only
            # the 1/s_u factor (descaled at the output copy).
            hT = []
            for i in range(NI):
                h_t = hp.tile([128, C], BF, tag=f"h{i}", name=f"hT{i}")
                for c0, cn in chunks:
                    csl = bass.ds(c0, cn)
                    g_ps = pg.tile([128, cn], F32, tag="g", name="g_ps")
                    u_ps = pu.tile([128, cn], F32, tag="u", name="u_ps")
                    for d in range(ND):
                        xsl = bass.ds(d * C + c0, cn)
                        nc.tensor.matmul(
                            g_ps[:],
                            wslice("g", i, d),
                            xT[:, xsl],
                            start=(d == 0),
                            stop=(d == ND - 1),
                        )
                    for d in range(ND):
                        xsl = bass.ds(d * C + c0, cn)
                        nc.tensor.matmul(
                            u_ps[:],
                            wslice("u", i, d),
                            xT[:, xsl],
                            start=(d == 0),
                            stop=(d == ND - 1),
                        )
                    s_sb = sp.tile([128, cn], F32, tag="s", name="s_sb")
                    nc.scalar.activation(
                        s_sb[:],
                        g_ps[:],
                        mybir.ActivationFunctionType.Silu,
                        scale=cst[:, bass.ds(0, 1)],
                    )
                    nc.vector.tensor_mul(h_t[:, csl], s_sb[:], u_ps[:])
                hT.append(h_t)

            # Phase 2: outT[dstrip] = Wd^T @ hidT, accumulated over all 16
            # intermediate strips. The PSUM->SBUF copy runs on ACT (idle in
            # phase 2) and applies the s_u*s_d dequant in its scale.
            for dd in range(ND):
                dsl = bass.ds(dd * 128, 128)
                for c0, cn in chunks:
                    csl = bass.ds(c0, cn)
                    o_ps = po.tile([128, cn], F32, tag="o", name="o_ps")
                    for i in range(NI):
                        nc.tensor.matmul(
                            o_ps[:],
                            wdslice(i, dd),
                            hT[i][:, csl],
                            start=(i == 0),
                            stop=(i == NI - 1),
                        )
                    o_sb = op.tile([128, cn], F32, tag="ob", name="o_sb")
                    if dd < ND - 1:
                        nc.vector.tensor_scalar_mul(
                            o_sb[:], o_ps[:], cst[:, bass.ds(1, 1)]
                        )
                        nc.sync.dma_start(outT_d[dsl, csl], o_sb[:])
                    else:
                        # final strip: copy + drain in two halves on parallel
                        # rings so the post-last-matmul tail is half as long
                        ch = cn // 2
                        h0, h1 = bass.ds(c0, ch), bass.ds(c0 + ch, cn - ch)
                        nc.vector.tensor_scalar_mul(
                            o_sb[:, bass.ds(0, ch)],
                            o_ps[:, bass.ds(0, ch)],
                            cst[:, bass.ds(1, 1)],
                        )
                        nc.scalar.dma_start(
                            outT_d[dsl, h0], o_sb[:, bass.ds(0, ch)]
                        )
                        nc.vector.tensor_scalar_mul(
                            o_sb[:, bass.ds(ch, cn - ch)],
                            o_ps[:, bass.ds(ch, cn - ch)],
                            cst[:, bass.ds(1, 1)],
                        )
                        nc.sync.dma_start(outT_d[dsl, h1], o_sb[:, bass.ds(ch, cn - ch)])

    nc.compile()
    return nc


def _get_program(C):
    if C not in _PROGRAM_CACHE:
        _PROGRAM_CACHE[C] = _build_program(C)
    return _PROGRAM_CACHE[C]


def _run(nc, in_maps, trace=False):
    from concourse.bass_utils import run_bass_kernel_spmd

    return run_bass_kernel_spmd(nc, in_maps, core_ids=list(range(E)), trace=trace)


def _quant8(w):
    # per-expert-tensor linear int8 quantization; returns (q, scale)
    s = float(np.abs(w).max()) / 127.0
    if s == 0.0:
        s = 1.0
    q = np.clip(np.rint(w / s), -127, 127).astype(np.int8)
    return q, s


def _pack_w(q, transpose):
    # transpose=True (wg/wu, [D, I]): -> [128, NI*D], free slot i*D + d*128 + q,
    #   block (i,d) = w[d*128:+128, i*128:+128]
    # transpose=False (wd, [I, D]): -> [128, ND*I] d-major, free slot
    #   d*I + i*128 + q, block (i,d) = w[i*128:+128, d*128:+128]
    if transpose:
        b = q.reshape(ND, 128, NI, 128).transpose(1, 2, 0, 3)  # p, i, d, q
    else:
        b = q.reshape(NI, 128, ND, 128).transpose(1, 2, 0, 3)  # p, d, i, q
    return np.ascontiguousarray(b.reshape(128, NI * D))


def _kernel_numpy(x, idx, w_gate, w_up, w_down):
    # exact fallback for pathological token skew (SBUF can't hold >~1536
    # tokens per expert); normal inputs never take this path
    out = np.zeros((T, D), dtype=np.float32)
    for e in range(E):
        m = idx == e
        if not m.any():
            continue
        g = x[m] @ w_gate[e]
        u = x[m] @ w_up[e]
        out[m] = (g / (1.0 + np.exp(-g)) * u) @ w_down[e]
    return out


def kernel(x, expert_indices, w_gate, w_up, w_down, _trace=False, _results=None):
    x = np.asarray(x)
    idx = np.asarray(expert_indices).astype(np.int64)
    counts = np.bincount(idx, minlength=E)
    C = int(max(128, -(-counts.max() // 4) * 4))
    if C > 1536:
        return _kernel_numpy(
            x, idx, np.asarray(w_gate), np.asarray(w_up), np.asarray(w_down)
        )

    nc = _get_program(C)

    order = np.argsort(idx, kind="stable")
    starts = np.zeros(E + 1, dtype=np.int64)
    np.cumsum(counts, out=starts[1:])

    bf16 = ml_dtypes.bfloat16
    in_maps = []
    for e in range(E):
        toks = order[starts[e] : starts[e + 1]]
        # xT packed: [128, ND*C]; [p, d*C+c] = x[tok c, d*128+p]
        xTg = np.zeros((128, ND, C), dtype=bf16)
        xTg[:, :, : len(toks)] = (
            x[toks].astype(bf16).T.reshape(ND, 128, len(toks)).transpose(1, 0, 2)
        )
        qg, sg = _quant8(np.asarray(w_gate[e]))
        qu, su = _quant8(np.asarray(w_up[e]))
        qd, sd = _quant8(np.asarray(w_down[e]))
        cst = np.empty((128, 2), dtype=np.float32)
        cst[:, 0] = sg
        cst[:, 1] = su * sd
        in_maps.append(
            {
                "xT": xTg.reshape(128, ND * C),
                "wg": _pack_w(qg, True),
                "wu": _pack_w(qu, True),
                "wd": _pack_w(qd, False),
                "cst": cst,
            }
        )

    res = _run(nc, in_maps, trace=_trace)
    if _results is not None:
        _results.append(res)

    out = np.zeros((T, D), dtype=np.float32)
    for e in range(E):
        toks = order[starts[e] : starts[e + 1]]
        outT = res.results[e]["outT"]  # [D, C] fp32
        out[toks] = outT[:, : len(toks)].T
    return out
